# revision 16
# baseline (speedup 1.0000x reference)
"""ForwardDiffusion (Ornstein-Uhlenbeck Euler-Maruyama) Trainium2 kernel.

Math: x_k = a*x_{k-1} + b*z_k with a = 1-THETA*DT, b = SIGMA0*sqrt(DT).
Host pre-scales the noise: zs_j = b * a^-j * z_j, so that
  x_k = a^k * (x0 + S_k),  S_k = sum_{j<=k} zs_j   (plain prefix sum).
Per 128-row k block (k on partitions, batch*length on free):
  - PE: S block via EXACT ones-triangular matmul (bf16 in, f32 psum) plus a
    rank-1 all-ones carry add; carry rows chain block to block unweighted.
  - ACT: psum -> sbuf bf16 copy of S (and the carry row for the next block).
  - DVE tensor_tensor (2x bf16 mode): y = x_bcast + S_bcast over all 8 batch
    rows at once ([128, 8192]).
  - scale by a^k: DVE tensor_scalar (4x mode) on most blocks, ACT
    activation(scale=apa) on ACT_TS_BLOCKS - balances the two engines.
  - out is bf16 in DRAM (halves HBM write traffic); host upcasts to f32.
k=0 plane is x itself - host writes it straight from the input.
Blocks 0-6 cover k=1..896; block 7 covers k=872..999 with a full-128-
partition DMA (partial-partition DMAs run ~16x slower), double-writing
rows 872..896 with equal values.
Outputs ride the SP ring; noise + x-broadcast ride the GpSimd ring.
Data parallel over batch: x sharded 8 ways, noise replicated, no collectives.
"""

import math
import os

import numpy as np
import ml_dtypes

import concourse.bass as bass
import concourse.bacc as bacc
import concourse.mybir as mybir
import concourse.tile as tile
from concourse.bass_utils import run_bass_kernel_spmd

# Problem config (hardcoded per harness contract)
THETA = 1.0
SIGMA0 = 0.5
DT = 0.001
BATCH = 64
LENGTH = 1024
STEPS = 1000           # real output rows per batch element (k = 0..999)
NK = STEPS - 1         # real noise rows (k = 1..999)
NCORES = 8
BPC = BATCH // NCORES  # batch rows per core = 8
NKB = 8                # 7 aligned k blocks + 1 overlapping final block
KROWS = STEPS
FREE = BPC * LENGTH    # 8192 free elems per output tile

A = 1.0 - THETA * DT           # 0.999
B = SIGMA0 * math.sqrt(DT)     # 0.0158113883...

F32 = mybir.dt.float32
BF16 = mybir.dt.bfloat16
NP_BF16 = ml_dtypes.bfloat16

ACT_TS_BLOCKS = (1, 2, 4, 5)   # blocks whose a^k scale runs on the ACT engine
                               # (never the last blocks: their DMAs are the tail)

_cache = {}


def _consts():
    """Host-precomputed constant tensors (exact in f64, then cast)."""
    if "consts" in _cache:
        return _cache["consts"]
    p = np.arange(128, dtype=np.float64)
    # per-partition output scale: apa[p, kb] = a^(kb*128 + p + 1)
    kb = np.arange(NKB, dtype=np.float64)
    apa = (A ** (kb[None, :] * 128.0 + p[:, None] + 1.0)).astype(np.float32)
    # last block: rows k = 872+p (872..999), full 128 partitions
    apa[:, 7] = (A ** (872.0 + p)).astype(np.float32)
    c = {"apa": apa}
    _cache["consts"] = c
    return c


def _build_nc():
    if "nc" in _cache:
        return _cache["nc"]
    nc = bacc.Bacc(
        "TRN2", target_bir_lowering=False, debug=False, num_devices=NCORES
    )
    x_p = nc.declare_dram_parameter("x", [BPC, LENGTH], BF16, isOutput=False)
    z_p = nc.declare_dram_parameter("noise", [NK, LENGTH], BF16, isOutput=False)
    apa_p = nc.declare_dram_parameter("apa", [128, NKB], F32, isOutput=False)
    out_p = nc.declare_dram_parameter("out", [BPC, KROWS, LENGTH], BF16, isOutput=True)

    HALF = 512  # one PSUM bank of f32 per matmul
    Copy = mybir.ActivationFunctionType.Copy

    with tile.TileContext(nc) as tc:
        with (
            tc.tile_pool(name="consts", bufs=1) as consts,
            tc.tile_pool(name="pers", bufs=1) as pers,
            tc.tile_pool(name="zt", bufs=4) as ztp,
            tc.tile_pool(name="cp", bufs=3) as cpp,
            tc.tile_pool(name="yp", bufs=2) as yp,
            tc.tile_pool(name="outp", bufs=4) as outp,
            tc.tile_pool(name="psc", bufs=2, space="PSUM") as pscp,
            tc.tile_pool(name="pscy", bufs=2, space="PSUM") as cyp,
        ):
            zt = [None] * NKB

            def emit_zt(kb, eng=None):
                r0 = kb * 128 if kb < 7 else NK - 128  # 871 for the last block
                t = ztp.tile([128, LENGTH], BF16, tag="zt")
                (eng or nc.gpsimd).dma_start(out=t[:], in_=z_p[r0 : r0 + 128, :])
                zt[kb] = t

            # zt0 first and alone on the SP ring: 2KB chunks ride all 16 DMA
            # engines, so the chain-critical load lands in ~1us
            emit_zt(0, eng=nc.sync)

            # triT / onesr are synthesized on device: a DMA of 256B-per-
            # partition chunks is descriptor-bound on ONE dma engine (~90ns
            # each = 11us for 128 rows); memset+affine_select takes <1us
            triT = consts.tile([128, 128], BF16, tag="triT")
            nc.gpsimd.memset(triT[:], 1.0)
            nc.gpsimd.affine_select(
                triT[:], triT[:], [[1, 128]], mybir.AluOpType.is_ge,
                0.0, base=0, channel_multiplier=-1,
            )
            onesr = consts.tile([1, 128], BF16, tag="onesr")
            nc.gpsimd.memset(onesr[:], 1.0)

            # apa is 32B per partition: split across three rings so the
            # descriptor-bound load takes ~4us instead of ~11us
            apa = consts.tile([128, NKB], F32, tag="apa")
            for r0, r1, eng in (
                (0, 43, nc.sync),
                (43, 86, nc.scalar),
                (86, 128, nc.gpsimd),
            ):
                eng.dma_start(out=apa[r0:r1, :], in_=apa_p[r0:r1, :])
            emit_zt(1)

            # all 8 batch rows broadcast to 128 partitions, one SWDGE DMA
            xball = pers.tile([128, FREE], BF16, tag="xball", name="xball")
            src = (
                x_p[:, :]
                .rearrange("(u b) l -> u b l", u=1)
                .broadcast_to((128, BPC, LENGTH))
            )
            xb3 = xball[:, :].rearrange("p (b l) -> p b l", l=LENGTH)
            nc.gpsimd.dma_start(out=xb3, in_=src)

            carry = [
                pers.tile([1, LENGTH], BF16, tag=f"cy{k}", name=f"cy{k}")
                for k in range(NKB - 1)
            ]

            for kb in range(NKB):
                if kb + 2 < NKB:
                    emit_zt(kb + 2)
                # main block first: it gates the DVE/ACT output chain, while
                # the carry only gates PE for block kb+1 (lots of slack)
                ps = pscp.tile([128, LENGTH], F32, tag="psc")
                for h in range(LENGTH // HALF):
                    sl = slice(h * HALF, (h + 1) * HALF)
                    # in-block prefix accumulation (exact ones-triangular)
                    nc.tensor.matmul(
                        ps[:, sl], triT[:, :], zt[kb][:, sl],
                        start=True, stop=(kb == 0),
                    )
                    if kb > 0:
                        # + carry row (all-ones rank-1)
                        nc.tensor.matmul(
                            ps[:, sl], onesr[:, :], carry[kb - 1][:, sl],
                            start=False, stop=True,
                        )
                # S block to SBUF bf16 (enables the DVE 2x packed mode)
                cp16 = cpp.tile([128, LENGTH], BF16, tag="cp16")
                nc.scalar.activation(cp16[:], ps[:, :], Copy)

                if kb < NKB - 1:
                    # carry row: S at the next block's seed k.
                    # kb<6: seed k=128(kb+1) (col 127 = sum of all 128 rows)
                    # kb=6: seed k=871 (col 102 = sum of rows 0..102)
                    kq, col = (128, 127) if kb < 6 else (103, 102)
                    cps = cyp.tile([1, LENGTH], F32, tag="cps", name="cps")
                    for h in range(LENGTH // HALF):
                        sl = slice(h * HALF, (h + 1) * HALF)
                        nc.tensor.matmul(
                            cps[:1, sl], triT[:kq, col : col + 1],
                            zt[kb][:kq, sl],
                            start=True, stop=(kb == 0),
                        )
                        if kb > 0:
                            nc.tensor.matmul(
                                cps[:1, sl], onesr[0:1, 0:1],
                                carry[kb - 1][:1, sl],
                                start=False, stop=True,
                            )
                    nc.scalar.activation(carry[kb][:], cps[:1, :], Copy)
                cbc = (
                    cp16[:, :]
                    .rearrange("p (u l) -> p u l", u=1)
                    .broadcast_to((128, BPC, LENGTH))
                )
                # y = x + S over all 8 batch rows (DVE 2x bf16 mode)
                yt = yp.tile([128, FREE], BF16, tag="yt")
                y3 = yt[:, :].rearrange("p (b l) -> p b l", l=LENGTH)
                nc.vector.tensor_tensor(y3, xb3, cbc, mybir.AluOpType.add)
                # out = y * a^k (per-partition scalar)
                ot = outp.tile([128, FREE], BF16, tag="ot")
                o3 = ot[:, :].rearrange("p (b l) -> p b l", l=LENGTH)
                if kb in ACT_TS_BLOCKS:
                    nc.scalar.activation(o3, y3, Copy, scale=apa[:, kb : kb + 1])
                else:
                    nc.vector.tensor_scalar(
                        o3, y3, apa[:, kb : kb + 1], None, mybir.AluOpType.mult
                    )
                # full 128-partition DMA always: partial-partition DMAs run
                # ~16x slower, so block 7 double-writes rows 872..896 instead
                k0 = 1 + kb * 128 if kb < 7 else KROWS - 128  # 872
                dst = out_p[:, k0 : k0 + 128, :].rearrange("b k l -> k b l")
                nc.sync.dma_start(out=dst, in_=o3)

    nc.compile()
    _cache["nc"] = nc
    return nc


def kernel(x: np.ndarray, noise: np.ndarray) -> np.ndarray:
    x = np.ascontiguousarray(np.asarray(x), dtype=np.float32)
    noise = np.asarray(noise)
    assert x.shape == (BATCH, LENGTH) and noise.shape == (NK, LENGTH)

    # host pre-scale: zs_j = b * a^-j * z_j  (j = 1..999), exact in f64
    j = np.arange(1, NK + 1, dtype=np.float64)
    zs = (noise.astype(np.float64) * (B * A ** (-j))[:, None]).astype(NP_BF16)
    xbf = x.astype(NP_BF16)

    nc = _build_nc()
    consts = _consts()
    in_maps = []
    for c in range(NCORES):
        m = dict(consts)
        m["noise"] = zs
        m["x"] = xbf[c * BPC : (c + 1) * BPC]
        in_maps.append(m)

    res = run_bass_kernel_spmd(nc, in_maps, core_ids=list(range(NCORES)))
    _cache["last_result"] = res
    out = np.concatenate(
        [
            res.results[i]["out"][:, :STEPS, :].astype(np.float32)
            for i in range(NCORES)
        ],
        axis=0,
    )
    out[:, 0, :] = x  # k=0 plane is the input itself, exact
    return np.ascontiguousarray(out)


def last_exec_time_ns():
    r = _cache.get("last_result")
    return None if r is None else r.exec_time_ns
